# revision 4
# baseline (speedup 1.0000x reference)
"""MinimumErrorRateLoss Trainium2 kernel (8 NeuronCores, data parallel).

Shards the flattened (batch*samples)=8192 sequence dimension across 8
cores (1024 sequences/core, globally sorted by hyp length, 8 segments of
128: SBUF partition = sequence row, free dim = segment-concatenated DP
columns, SEGW=260 per segment for alignment).

Row-DP over hyp steps in "u-space" (u_t[j] = y_t[j] + active_steps, where
y_t[j] = j - row_t[j] is the deramped Levenshtein row). Per hyp step t:
    eq1[g][j] = (ref[j] == h_t[g]) + act_t[g]      8x tensor_scalar
                (act=0 once the hyp ended and h_t is the -1 pad token,
                 which makes the whole step an exact no-op: u' = u,
                 eliminating the predicated freeze entirely)
    A[j]      = u[j-1] + eq1[j]                    1 flat tensor_tensor
    S_g       = cummax(A_g)                        8x per-segment
                                                   tensor_tensor_scan
                (8 narrow scans are ~5x faster than one wide scan)
    u'        = max(u, S)                          1 flat tensor_tensor
dist = rl + hl - u_H[rl]; extraction via one-hot at rl + reduce; the
trivial softmax epilogue runs on host in float64.
"""

import numpy as np
from contextlib import ExitStack

import concourse.bass as bass
import concourse.mybir as mybir
import concourse.tile as tile
from concourse.vector_clock import ScopedClock, VectorClock


def _split_drain_and_barrier(self, tick_clock, wait_clock):
    """Replacement for TileContext._drain_and_barrier: the walrus build in
    this container rejects instructions carrying more than one sync wait,
    so emit one single-wait drain per outstanding proc instead of a single
    drain waiting on every semaphore."""
    gc = tick_clock.global_clock
    nprocs = len(gc)
    for p in range(nprocs):
        t = gc[p]
        if t <= 0:
            continue
        vc = VectorClock([0] * nprocs)
        vc.require_at_least(p, t)
        d = self.nc.sync.drain()
        wait_clock.add_sem_waits(d.ins, ScopedClock({None: vc}))
    self.nc.all_engine_barrier()
    assert self.sems is not None
    popped = self.nc._tile_sem_poison_stack.pop()
    assert popped is self._sem_poison
    self.nc.clear_and_free_semaphores(list(self.sems.allocated().values()))
    self.nc.all_engine_barrier()


tile.TileContext._drain_and_barrier = _split_drain_and_barrier

# Problem constants (hardcoded per contract)
B, S = 128, 64          # batch, samples
RL, H = 256, 256        # ref len, hyp len
NCORES = 8
NPC = (B // NCORES) * S  # 1024 sequences per core
G = NPC // 128           # 8 segments of 128 sequences
SEGW = 260               # cols 0..257 used (j=0..257), 258/259 spacer
W = G * SEGW             # 2080
F16 = mybir.dt.float16
F32 = mybir.dt.float32
AO = mybir.AluOpType
TBUCKET = 16


class _Runner:
    """Compiled SPMD executable for a Bass module (mirrors
    bass2jax.run_bass_via_pjrt, but cached + device-resident timing)."""

    def __init__(self, nc, n_cores):
        import jax
        from jax.sharding import Mesh, PartitionSpec
        from jax.experimental.shard_map import shard_map
        from concourse.bass2jax import (_bass_exec_p, install_neuronx_cc_hook,
                                        partition_id_tensor)

        install_neuronx_cc_hook()
        self.nc, self.n_cores = nc, n_cores
        part_name = (nc.partition_id_tensor.name
                     if nc.partition_id_tensor else None)
        in_names, out_names, out_avals, zero_outs = [], [], [], []
        for alloc in nc.m.functions[0].allocations:
            if not isinstance(alloc, mybir.MemoryLocationSet):
                continue
            name = alloc.memorylocations[0].name
            if alloc.kind == "ExternalInput":
                if name != part_name:
                    in_names.append(name)
            elif alloc.kind == "ExternalOutput":
                out_names.append(name)
                shape = tuple(alloc.tensor_shape)
                dtype = mybir.dt.np(alloc.dtype)
                out_avals.append(jax.core.ShapedArray(shape, dtype))
                zero_outs.append(np.zeros(shape, dtype))
        self.in_names, self.out_names = list(in_names), out_names
        self.out_avals, self.zero_outs = out_avals, zero_outs
        n_params, n_outs = len(in_names), len(out_names)
        all_names = in_names + out_names
        if part_name is not None:
            all_names = all_names + [part_name]

        def _body(*args):
            operands = list(args)
            if part_name is not None:
                operands.append(partition_id_tensor())
            outs = _bass_exec_p.bind(
                *operands,
                out_avals=tuple(out_avals),
                in_names=tuple(all_names),
                out_names=tuple(out_names),
                lowering_input_output_aliases=(),
                sim_require_finite=True,
                sim_require_nnan=True,
                nc=nc,
            )
            return tuple(outs)

        devices = jax.devices()[:n_cores]
        self.mesh = Mesh(np.asarray(devices), ("core",))
        in_specs = (PartitionSpec("core"),) * (n_params + n_outs)
        out_specs = (PartitionSpec("core"),) * n_outs
        self.sharded = jax.jit(
            shard_map(_body, mesh=self.mesh, in_specs=in_specs,
                      out_specs=out_specs, check_rep=False),
            donate_argnums=tuple(range(n_params, n_params + n_outs)),
            keep_unused=True,
        )
        self.jax = jax

    def place_inputs(self, in_maps):
        import jax
        from jax.sharding import NamedSharding, PartitionSpec
        sh = NamedSharding(self.mesh, PartitionSpec("core"))
        concat = [
            np.concatenate([np.asarray(m[name]) for m in in_maps], axis=0)
            for name in self.in_names
        ]
        return [jax.device_put(a, sh) for a in concat]

    def zeros(self):
        import jax
        from jax.sharding import NamedSharding, PartitionSpec
        sh = NamedSharding(self.mesh, PartitionSpec("core"))
        return [
            jax.device_put(
                np.zeros((self.n_cores * z.shape[0], *z.shape[1:]), z.dtype), sh)
            for z in self.zero_outs
        ]

    def __call__(self, dev_inputs):
        return self.sharded(*dev_inputs, *self.zeros())

    def gather(self, out_arrs):
        res = []
        for c in range(self.n_cores):
            res.append({
                name: np.asarray(out_arrs[i]).reshape(
                    self.n_cores, *self.out_avals[i].shape)[c]
                for i, name in enumerate(self.out_names)
            })
        return res


def _build_nc(tdrop: tuple):
    """tdrop[g] = step after which segment g is fully frozen (multiple of
    TBUCKET; 256 = never drops). Nondecreasing (globally sorted layout)."""
    WIN = G * RL + 2 * G * H + G + SEGW
    nc = bass.Bass()
    inp = nc.declare_dram_parameter("inp", [128, WIN], F32, isOutput=False)
    y_out = nc.declare_dram_parameter("yrm", [128, G], F32, isOutput=True)

    with ExitStack() as ctx:
        tc = ctx.enter_context(tile.TileContext(nc))
        pool = ctx.enter_context(tc.tile_pool(name="main", bufs=1))

        inp_sb = pool.tile([128, WIN], F32, tag="inp")
        nc.sync.dma_start(inp_sb[:], inp[:])
        ref_f32 = inp_sb[:, 0:G * RL]
        o1 = G * RL
        hyp3 = inp_sb[:, o1:o1 + G * H].rearrange("p (g t) -> p g t",
                                                  g=G, t=H)
        o2 = o1 + G * H
        act3 = inp_sb[:, o2:o2 + G * H].rearrange("p (g t) -> p g t",
                                                  g=G, t=H)
        o3 = o2 + G * H
        rl_sb = inp_sb[:, o3:o3 + G]
        iota = inp_sb[:, o3 + G:o3 + G + SEGW]

        # ref cast to fp16, one 256-col block per SEGW segment (cols 0..255
        # of the eq1 layout, which stores eq1 for j = c+1)
        REF = pool.tile([128, W], F16, tag="ref")
        R3 = REF[:].rearrange("p (g c) -> p g c", g=G, c=SEGW)
        nc.vector.tensor_copy(
            R3[:, :, 0:RL],
            ref_f32.rearrange("p (g c) -> p g c", g=G, c=RL))

        # DP state and temporaries
        U1 = pool.tile([128, W], F16, tag="u1")
        U2 = pool.tile([128, W], F16, tag="u2")
        E1 = pool.tile([128, W], F16, tag="e1")
        AZ = pool.tile([128, W], F16, tag="az")
        Z0 = pool.tile([128, W], F16, tag="z0")
        RM = pool.tile([128, W], F16, tag="rm")
        yrm_sb = pool.tile([128, G], F32, tag="yrm")

        for t_ in (U1, U2, Z0, AZ):
            nc.vector.memset(t_[:], 0.0)
        nc.vector.memset(E1[:], 0.0)
        # poison eq1 spacer cols so the flat shifted add leaves
        # A[g][0] <= -1024 at every segment head (scan boundary)
        E3 = E1[:].rearrange("p (g c) -> p g c", g=G, c=SEGW)
        nc.vector.memset(E3[:, :, RL:SEGW], -2048.0)

        # extraction one-hot at col rl per segment (RM[g, j] = (j == rl))
        RM3 = RM[:].rearrange("p (g c) -> p g c", g=G, c=SEGW)
        nc.vector.memset(RM[:], 0.0)
        for g in range(G):
            nc.vector.tensor_scalar(RM3[:, g, 0:SEGW], iota,
                                    rl_sb[:, g:g + 1], None, AO.is_equal)

        Uc, Vc = U1, U2
        for t in range(1, H + 1):
            g0 = 0
            while g0 < G and tdrop[g0] < t:
                g0 += 1
            off = g0 * SEGW
            # eq1[g][c] (c = j-1): (ref == h_t) + act_t   [0 when frozen]
            for g in range(g0, G):
                nc.vector.tensor_scalar(
                    E3[:, g, 0:RL], R3[:, g, 0:RL],
                    hyp3[:, g, t - 1:t], act3[:, g, t - 1:t],
                    AO.is_equal, AO.add)
            # A[j] = u[j-1] + eq1[j]   (flat, spacers poisoned via E1)
            nc.vector.tensor_tensor(
                AZ[:, off + 1:W], Uc[:, off:W - 1], E1[:, off:W - 1], AO.add)
            # S_g = cummax(A_g) over j = 0..257 (col0 <= -1024 resets)
            for g in range(g0, G):
                nc.vector.tensor_tensor_scan(
                    Vc[:, g * SEGW:g * SEGW + RL + 2],
                    Z0[:, g * SEGW:g * SEGW + RL + 2],
                    AZ[:, g * SEGW:g * SEGW + RL + 2],
                    0.0, AO.add, AO.max)
            # u' = max(u, S)
            nc.vector.tensor_tensor(
                Vc[:, off:W], Vc[:, off:W], Uc[:, off:W], AO.max)
            Uc, Vc = Vc, Uc

        # extraction: u_H at col rl per segment (u stored at col j)
        nc.vector.tensor_tensor(AZ[:], Uc[:], RM[:], AO.mult)
        A3 = AZ[:].rearrange("p (g c) -> p g c", g=G, c=SEGW)
        for g in range(G):
            nc.vector.tensor_reduce(yrm_sb[:, g:g + 1], A3[:, g, :],
                                    mybir.AxisListType.X, AO.add)
        nc.sync.dma_start(y_out[:], yrm_sb[:])

    return nc


_NC_CACHE = {}


def _get_nc(tdrop):
    if tdrop not in _NC_CACHE:
        _NC_CACHE[tdrop] = _build_nc(tdrop)
    return _NC_CACHE[tdrop]


_RUNNER_CACHE = {}


def _get_runner(nc):
    key = id(nc)
    if key not in _RUNNER_CACHE:
        _RUNNER_CACHE[key] = _Runner(nc, NCORES)
    return _RUNNER_CACHE[key]


def _lens(tok, axis):
    is_eos = tok == 0
    has = is_eos.any(axis=axis)
    idx = np.argmax(is_eos, axis=axis)
    return np.where(has, idx + 1, tok.shape[axis])


def _prep(log_probs, ref, hyp):
    """Global sort by hyp length; build per-core DMA images."""
    refT = np.ascontiguousarray(ref.T).astype(np.float32)       # (B, RL)
    # (B*S, H) with sequence index s = b*S + k (matches reference flatten)
    hypF = np.ascontiguousarray(
        hyp.transpose(1, 2, 0)).reshape(B * S, H).astype(np.float32)
    hls = _lens(hypF, 1).astype(np.int64)                       # (B*S,)
    rlens = _lens(np.asarray(ref), 0).astype(np.int64)          # (B,)

    order = np.argsort(hls, kind="stable")                      # global sort
    hyp_s = hypF[order]
    hls_s = hls[order]
    rl_s = np.repeat(rlens, S)[order].astype(np.float32)
    ref_s = refT[order // S]                                    # (B*S, RL)

    # pad ended hyp positions with a never-matching token
    tidx = np.arange(H)[None, :]
    act_s = (tidx < hls_s[:, None]).astype(np.float32)
    hyp_s = np.where(act_s > 0, hyp_s, -1.0).astype(np.float32)

    iota = np.broadcast_to(np.arange(SEGW, dtype=np.float32), (128, SEGW))
    in_maps = []
    for c in range(NCORES):
        sel = order  # bookkeeping only; data already sorted
        idx = np.arange(c, B * S, NCORES)                       # ranks of core c
        # rank r -> seg = (r//8)//128, row = (r//8)%128
        q = idx // NCORES
        segs, rows = q // 128, q % 128
        refc = np.zeros((128, G, RL), np.float32)
        hypc = np.zeros((128, G, H), np.float32)
        actc = np.zeros((128, G, H), np.float32)
        rlc = np.zeros((128, G), np.float32)
        refc[rows, segs] = ref_s[idx]
        hypc[rows, segs] = hyp_s[idx]
        actc[rows, segs] = act_s[idx]
        rlc[rows, segs] = rl_s[idx]
        in_maps.append({"inp": np.ascontiguousarray(np.concatenate(
            [refc.reshape(128, G * RL), hypc.reshape(128, G * H),
             actc.reshape(128, G * H), rlc, iota], axis=1))})
    seg_max = np.zeros(G, dtype=np.int64)
    hq = hls_s.reshape(G, 128 * NCORES)
    seg_max = hq.max(axis=1)
    tdrop = tuple(int(min(H, -(-m // TBUCKET) * TBUCKET)) for m in seg_max)
    return in_maps, order, tdrop


def _epilogue(youts, order, log_probs, ref, hyp):
    ref_np = np.asarray(ref)
    rlens_b = _lens(ref_np, 0).astype(np.float64)               # (B,)
    hypF = np.asarray(hyp).transpose(1, 2, 0).reshape(B * S, H)
    hls = _lens(hypF, 1).astype(np.float64)                     # (B*S,)
    rl_all = np.repeat(rlens_b, S)                              # (B*S,)

    u_final = np.empty(B * S, dtype=np.float64)
    for c in range(NCORES):
        idx = np.arange(c, B * S, NCORES)
        q = idx // NCORES
        segs, rows = q // 128, q % 128
        u_final[order[idx]] = youts[c][rows, segs]
    dist = rl_all + hls - u_final
    er = (dist / rl_all).reshape(B, S)
    er = er - er.mean(axis=1, keepdims=True)
    lp = np.asarray(log_probs).astype(np.float64)
    sm = np.exp(lp - lp.max(axis=1, keepdims=True))
    sm /= sm.sum(axis=1, keepdims=True)
    return np.float32((er * sm).mean())


def kernel(log_probs, ref, hyp, _sim=False):
    lp, ref, hyp = np.asarray(log_probs), np.asarray(ref), np.asarray(hyp)
    in_maps, order, tdrop = _prep(lp, ref, hyp)
    nc = _get_nc(tdrop)
    if _sim:
        from concourse.bass_interp import CoreSim
        youts = []
        for c in range(NCORES):
            sim = CoreSim(nc, trace=False)
            for k, v in in_maps[c].items():
                sim.tensor(k)[:] = v
            sim.simulate()
            youts.append(np.array(sim.tensor("yrm")))
        return _epilogue(youts, order, lp, ref, hyp)

    runner = _get_runner(nc)
    dev_in = runner.place_inputs(in_maps)
    results = runner.gather(runner(dev_in))
    youts = [results[c]["yrm"] for c in range(NCORES)]
    return _epilogue(youts, order, lp, ref, hyp)


# revision 6
# speedup vs baseline: 1.2614x; 1.2614x over previous
"""MinimumErrorRateLoss Trainium2 kernel (8 NeuronCores, data parallel).

Shards the flattened (batch*samples)=8192 sequence dimension across 8
cores (1024 sequences/core, globally sorted by hyp length, 8 segments of
128: SBUF partition = sequence row, free dim = segment-concatenated DP
columns, SEGW=260 per segment for alignment).

Row-DP over hyp steps in "u-space" (u_t[j] = y_t[j] + active_steps, where
y_t[j] = j - row_t[j] is the deramped Levenshtein row). Per hyp step t:
    eq1[g][j] = (ref[j] == h_t[g]) + act_t[g]      8x tensor_scalar
                (act=0 once the hyp ended and h_t is the -1 pad token,
                 which makes the whole step an exact no-op: u' = u,
                 eliminating the predicated freeze entirely)
    A[j]      = u[j-1] + eq1[j]                    1 flat tensor_tensor
    S_g       = cummax(A_g)                        8x per-segment
                                                   tensor_tensor_scan
                (8 narrow scans are ~5x faster than one wide scan)
    u'        = max(u, S)                          1 flat tensor_tensor
dist = rl + hl - u_H[rl]; extraction via one-hot at rl + reduce; the
trivial softmax epilogue runs on host in float64.
"""

import numpy as np
from contextlib import ExitStack

import concourse.bass as bass
import concourse.mybir as mybir
import concourse.tile as tile
from concourse.vector_clock import ScopedClock, VectorClock


def _split_drain_and_barrier(self, tick_clock, wait_clock):
    """Replacement for TileContext._drain_and_barrier: the walrus build in
    this container rejects instructions carrying more than one sync wait,
    so emit one single-wait drain per outstanding proc instead of a single
    drain waiting on every semaphore."""
    gc = tick_clock.global_clock
    nprocs = len(gc)
    for p in range(nprocs):
        t = gc[p]
        if t <= 0:
            continue
        vc = VectorClock([0] * nprocs)
        vc.require_at_least(p, t)
        d = self.nc.sync.drain()
        wait_clock.add_sem_waits(d.ins, ScopedClock({None: vc}))
    self.nc.all_engine_barrier()
    assert self.sems is not None
    popped = self.nc._tile_sem_poison_stack.pop()
    assert popped is self._sem_poison
    self.nc.clear_and_free_semaphores(list(self.sems.allocated().values()))
    self.nc.all_engine_barrier()


tile.TileContext._drain_and_barrier = _split_drain_and_barrier

# Problem constants (hardcoded per contract)
B, S = 128, 64          # batch, samples
RL, H = 256, 256        # ref len, hyp len
NCORES = 8
NPC = (B // NCORES) * S  # 1024 sequences per core
G = NPC // 128           # 8 segments of 128 sequences
SEGW = 260               # cols 0..257 used (j=0..257), 258/259 spacer
W = G * SEGW             # 2080
F16 = mybir.dt.float16
F32 = mybir.dt.float32
AO = mybir.AluOpType
TBUCKET = 16


class _Runner:
    """Compiled SPMD executable for a Bass module (mirrors
    bass2jax.run_bass_via_pjrt, but cached + device-resident timing)."""

    def __init__(self, nc, n_cores):
        import jax
        from jax.sharding import Mesh, PartitionSpec
        from jax.experimental.shard_map import shard_map
        from concourse.bass2jax import (_bass_exec_p, install_neuronx_cc_hook,
                                        partition_id_tensor)

        install_neuronx_cc_hook()
        self.nc, self.n_cores = nc, n_cores
        part_name = (nc.partition_id_tensor.name
                     if nc.partition_id_tensor else None)
        in_names, out_names, out_avals, zero_outs = [], [], [], []
        for alloc in nc.m.functions[0].allocations:
            if not isinstance(alloc, mybir.MemoryLocationSet):
                continue
            name = alloc.memorylocations[0].name
            if alloc.kind == "ExternalInput":
                if name != part_name:
                    in_names.append(name)
            elif alloc.kind == "ExternalOutput":
                out_names.append(name)
                shape = tuple(alloc.tensor_shape)
                dtype = mybir.dt.np(alloc.dtype)
                out_avals.append(jax.core.ShapedArray(shape, dtype))
                zero_outs.append(np.zeros(shape, dtype))
        self.in_names, self.out_names = list(in_names), out_names
        self.out_avals, self.zero_outs = out_avals, zero_outs
        n_params, n_outs = len(in_names), len(out_names)
        all_names = in_names + out_names
        if part_name is not None:
            all_names = all_names + [part_name]

        def _body(*args):
            operands = list(args)
            if part_name is not None:
                operands.append(partition_id_tensor())
            outs = _bass_exec_p.bind(
                *operands,
                out_avals=tuple(out_avals),
                in_names=tuple(all_names),
                out_names=tuple(out_names),
                lowering_input_output_aliases=(),
                sim_require_finite=True,
                sim_require_nnan=True,
                nc=nc,
            )
            return tuple(outs)

        devices = jax.devices()[:n_cores]
        self.mesh = Mesh(np.asarray(devices), ("core",))
        in_specs = (PartitionSpec("core"),) * (n_params + n_outs)
        out_specs = (PartitionSpec("core"),) * n_outs
        self.sharded = jax.jit(
            shard_map(_body, mesh=self.mesh, in_specs=in_specs,
                      out_specs=out_specs, check_rep=False),
            donate_argnums=tuple(range(n_params, n_params + n_outs)),
            keep_unused=True,
        )
        self.jax = jax

    def place_inputs(self, in_maps):
        import jax
        from jax.sharding import NamedSharding, PartitionSpec
        sh = NamedSharding(self.mesh, PartitionSpec("core"))
        concat = [
            np.concatenate([np.asarray(m[name]) for m in in_maps], axis=0)
            for name in self.in_names
        ]
        return [jax.device_put(a, sh) for a in concat]

    def zeros(self):
        import jax
        from jax.sharding import NamedSharding, PartitionSpec
        sh = NamedSharding(self.mesh, PartitionSpec("core"))
        return [
            jax.device_put(
                np.zeros((self.n_cores * z.shape[0], *z.shape[1:]), z.dtype), sh)
            for z in self.zero_outs
        ]

    def __call__(self, dev_inputs):
        return self.sharded(*dev_inputs, *self.zeros())

    def gather(self, out_arrs):
        res = []
        for c in range(self.n_cores):
            res.append({
                name: np.asarray(out_arrs[i]).reshape(
                    self.n_cores, *self.out_avals[i].shape)[c]
                for i, name in enumerate(self.out_names)
            })
        return res


def _build_nc(tdrop: tuple):
    """tdrop[g] = step after which segment g is fully frozen (multiple of
    TBUCKET; 256 = never drops). Nondecreasing (globally sorted layout)."""
    WIN = G * RL + 2 * G * H + G + SEGW
    nc = bass.Bass()
    inp = nc.declare_dram_parameter("inp", [128, WIN], F32, isOutput=False)
    y_out = nc.declare_dram_parameter("yrm", [128, G], F32, isOutput=True)

    with ExitStack() as ctx:
        tc = ctx.enter_context(tile.TileContext(nc))
        pool = ctx.enter_context(tc.tile_pool(name="main", bufs=1))

        inp_sb = pool.tile([128, WIN], F32, tag="inp")
        nc.sync.dma_start(inp_sb[:], inp[:])
        ref_f32 = inp_sb[:, 0:G * RL]
        o1 = G * RL
        hyp3 = inp_sb[:, o1:o1 + G * H].rearrange("p (g t) -> p g t",
                                                  g=G, t=H)
        o2 = o1 + G * H
        act3 = inp_sb[:, o2:o2 + G * H].rearrange("p (g t) -> p g t",
                                                  g=G, t=H)
        o3 = o2 + G * H
        rl_sb = inp_sb[:, o3:o3 + G]
        iota = inp_sb[:, o3 + G:o3 + G + SEGW]

        # ref cast to fp16, one 256-col block per SEGW segment (cols 0..255
        # of the eq1 layout, which stores eq1 for j = c+1)
        REF = pool.tile([128, W], F16, tag="ref")
        R3 = REF[:].rearrange("p (g c) -> p g c", g=G, c=SEGW)
        nc.vector.tensor_copy(
            R3[:, :, 0:RL],
            ref_f32.rearrange("p (g c) -> p g c", g=G, c=RL))

        # DP state and temporaries (eq1 double-buffered per 8-step chunk)
        CH = 8
        U1 = pool.tile([128, W], F16, tag="u1")
        U2 = pool.tile([128, W], F16, tag="u2")
        EA = pool.tile([128, CH * W], F16, tag="ea")
        EB = pool.tile([128, CH * W], F16, tag="eb")
        AZ = pool.tile([128, W], F16, tag="az")
        Z0 = pool.tile([128, W], F16, tag="z0")
        RM = pool.tile([128, W], F16, tag="rm")
        yrm_sb = pool.tile([128, G], F32, tag="yrm")

        for t_ in (U1, U2, Z0, AZ):
            nc.vector.memset(t_[:], 0.0)
        # poison eq1 spacer cols so the flat shifted add leaves
        # A[g][0] <= -1024 at every segment head (scan boundary)
        for Et in (EA, EB):
            nc.vector.memset(Et[:], 0.0)
            E4 = Et[:].rearrange("p (k g c) -> p k g c", k=CH, g=G, c=SEGW)
            nc.vector.memset(E4[:, :, :, RL:SEGW], -2048.0)

        # extraction one-hot at col rl per segment (RM[g, j] = (j == rl))
        RM3 = RM[:].rearrange("p (g c) -> p g c", g=G, c=SEGW)
        nc.vector.memset(RM[:], 0.0)
        for g in range(G):
            nc.vector.tensor_scalar(RM3[:, g, 0:SEGW], iota,
                                    rl_sb[:, g:g + 1], None, AO.is_equal)

        def active_g0(t):
            g0 = 0
            while g0 < G and tdrop[g0] < t:
                g0 += 1
            return g0

        def emit_ts_burst(Et, t0):
            """eq1 for steps t0..t0+CH-1 into Et (all independent)."""
            E4 = Et[:].rearrange("p (k g c) -> p k g c", k=CH, g=G, c=SEGW)
            for k in range(CH):
                t = t0 + k
                if t > H:
                    break
                for g in range(active_g0(t), G):
                    nc.vector.tensor_scalar(
                        E4[:, k, g, 0:RL], R3[:, g, 0:RL],
                        hyp3[:, g, t - 1:t], act3[:, g, t - 1:t],
                        AO.is_equal, AO.add)

        emit_ts_burst(EA, 1)
        Uc, Vc = U1, U2
        for t in range(1, H + 1):
            k = (t - 1) % CH
            if k == 0 and t + CH <= H + 1:
                emit_ts_burst(EB if ((t - 1) // CH) % 2 == 0 else EA,
                              t + CH)
            Et = EA if ((t - 1) // CH) % 2 == 0 else EB
            Ek = Et[:, k * W:(k + 1) * W]
            g0 = active_g0(t)
            # 4 streams of 2 segments, phases grouped (g4 schedule):
            # stream deps land >=3 blocks back, hiding drain latency
            streams = []
            for s in range(4):
                lo = max(2 * s, g0)
                if lo < 2 * s + 2:
                    streams.append((lo * SEGW, (2 * s + 2) * SEGW))
            # A[j] = u[j-1] + eq1[j]
            for (lo, hi) in streams:
                nc.vector.tensor_tensor(
                    AZ[:, lo + 1:hi], Uc[:, lo:hi - 1], Ek[:, lo:hi - 1],
                    AO.add)
            # S_g = cummax(A_g) over j = 0..257 (col0 <= -1024 resets)
            for g in range(g0, G):
                nc.vector.tensor_tensor_scan(
                    Vc[:, g * SEGW:g * SEGW + RL + 2],
                    Z0[:, g * SEGW:g * SEGW + RL + 2],
                    AZ[:, g * SEGW:g * SEGW + RL + 2],
                    0.0, AO.add, AO.max)
            # u' = max(u, S)
            for (lo, hi) in streams:
                nc.vector.tensor_tensor(
                    Vc[:, lo:hi], Vc[:, lo:hi], Uc[:, lo:hi], AO.max)
            Uc, Vc = Vc, Uc

        # extraction: u_H at col rl per segment (u stored at col j)
        nc.vector.tensor_tensor(AZ[:], Uc[:], RM[:], AO.mult)
        A3 = AZ[:].rearrange("p (g c) -> p g c", g=G, c=SEGW)
        for g in range(G):
            nc.vector.tensor_reduce(yrm_sb[:, g:g + 1], A3[:, g, :],
                                    mybir.AxisListType.X, AO.add)
        nc.sync.dma_start(y_out[:], yrm_sb[:])

    return nc


_NC_CACHE = {}


def _get_nc(tdrop):
    if tdrop not in _NC_CACHE:
        _NC_CACHE[tdrop] = _build_nc(tdrop)
    return _NC_CACHE[tdrop]


_RUNNER_CACHE = {}


def _get_runner(nc):
    key = id(nc)
    if key not in _RUNNER_CACHE:
        _RUNNER_CACHE[key] = _Runner(nc, NCORES)
    return _RUNNER_CACHE[key]


def _lens(tok, axis):
    is_eos = tok == 0
    has = is_eos.any(axis=axis)
    idx = np.argmax(is_eos, axis=axis)
    return np.where(has, idx + 1, tok.shape[axis])


def _prep(log_probs, ref, hyp):
    """Global sort by hyp length; build per-core DMA images."""
    refT = np.ascontiguousarray(ref.T).astype(np.float32)       # (B, RL)
    # (B*S, H) with sequence index s = b*S + k (matches reference flatten)
    hypF = np.ascontiguousarray(
        hyp.transpose(1, 2, 0)).reshape(B * S, H).astype(np.float32)
    hls = _lens(hypF, 1).astype(np.int64)                       # (B*S,)
    rlens = _lens(np.asarray(ref), 0).astype(np.int64)          # (B,)

    order = np.argsort(hls, kind="stable")                      # global sort
    hyp_s = hypF[order]
    hls_s = hls[order]
    rl_s = np.repeat(rlens, S)[order].astype(np.float32)
    ref_s = refT[order // S]                                    # (B*S, RL)

    # pad ended hyp positions with a never-matching token
    tidx = np.arange(H)[None, :]
    act_s = (tidx < hls_s[:, None]).astype(np.float32)
    hyp_s = np.where(act_s > 0, hyp_s, -1.0).astype(np.float32)

    iota = np.broadcast_to(np.arange(SEGW, dtype=np.float32), (128, SEGW))
    in_maps = []
    for c in range(NCORES):
        sel = order  # bookkeeping only; data already sorted
        idx = np.arange(c, B * S, NCORES)                       # ranks of core c
        # rank r -> seg = (r//8)//128, row = (r//8)%128
        q = idx // NCORES
        segs, rows = q // 128, q % 128
        refc = np.zeros((128, G, RL), np.float32)
        hypc = np.zeros((128, G, H), np.float32)
        actc = np.zeros((128, G, H), np.float32)
        rlc = np.zeros((128, G), np.float32)
        refc[rows, segs] = ref_s[idx]
        hypc[rows, segs] = hyp_s[idx]
        actc[rows, segs] = act_s[idx]
        rlc[rows, segs] = rl_s[idx]
        in_maps.append({"inp": np.ascontiguousarray(np.concatenate(
            [refc.reshape(128, G * RL), hypc.reshape(128, G * H),
             actc.reshape(128, G * H), rlc, iota], axis=1))})
    seg_max = np.zeros(G, dtype=np.int64)
    hq = hls_s.reshape(G, 128 * NCORES)
    seg_max = hq.max(axis=1)
    tdrop = tuple(int(min(H, -(-m // TBUCKET) * TBUCKET)) for m in seg_max)
    return in_maps, order, tdrop


def _epilogue(youts, order, log_probs, ref, hyp):
    ref_np = np.asarray(ref)
    rlens_b = _lens(ref_np, 0).astype(np.float64)               # (B,)
    hypF = np.asarray(hyp).transpose(1, 2, 0).reshape(B * S, H)
    hls = _lens(hypF, 1).astype(np.float64)                     # (B*S,)
    rl_all = np.repeat(rlens_b, S)                              # (B*S,)

    u_final = np.empty(B * S, dtype=np.float64)
    for c in range(NCORES):
        idx = np.arange(c, B * S, NCORES)
        q = idx // NCORES
        segs, rows = q // 128, q % 128
        u_final[order[idx]] = youts[c][rows, segs]
    dist = rl_all + hls - u_final
    er = (dist / rl_all).reshape(B, S)
    er = er - er.mean(axis=1, keepdims=True)
    lp = np.asarray(log_probs).astype(np.float64)
    sm = np.exp(lp - lp.max(axis=1, keepdims=True))
    sm /= sm.sum(axis=1, keepdims=True)
    return np.float32((er * sm).mean())


def kernel(log_probs, ref, hyp, _sim=False):
    lp, ref, hyp = np.asarray(log_probs), np.asarray(ref), np.asarray(hyp)
    in_maps, order, tdrop = _prep(lp, ref, hyp)
    nc = _get_nc(tdrop)
    if _sim:
        from concourse.bass_interp import CoreSim
        youts = []
        for c in range(NCORES):
            sim = CoreSim(nc, trace=False)
            for k, v in in_maps[c].items():
                sim.tensor(k)[:] = v
            sim.simulate()
            youts.append(np.array(sim.tensor("yrm")))
        return _epilogue(youts, order, lp, ref, hyp)

    runner = _get_runner(nc)
    dev_in = runner.place_inputs(in_maps)
    results = runner.gather(runner(dev_in))
    youts = [results[c]["yrm"] for c in range(NCORES)]
    return _epilogue(youts, order, lp, ref, hyp)


# revision 13
# speedup vs baseline: 1.7427x; 1.3816x over previous
"""MinimumErrorRateLoss Trainium2 kernel (8 NeuronCores, data parallel).

Shards the flattened (batch*samples)=8192 sequence dimension across 8
cores (1024 sequences/core, globally sorted by hyp length, 8 segments of
128: SBUF partition = sequence row, free dim = segment-concatenated DP
columns, SEGW=260 per segment for alignment).

Row-DP over hyp steps in "u-space" (u_t[j] = y_t[j] + active_steps, where
y_t[j] = j - row_t[j] is the deramped Levenshtein row). Per hyp step t:
    eq1[g][j] = (ref[j] == h_t[g]) + act_t[g]      8x tensor_scalar
                (act=0 once the hyp ended and h_t is the -1 pad token,
                 which makes the whole step an exact no-op: u' = u,
                 eliminating the predicated freeze entirely)
    A[j]      = u[j-1] + eq1[j]                    1 flat tensor_tensor
    S_g       = cummax(A_g)                        8x per-segment
                                                   tensor_tensor_scan
                (8 narrow scans are ~5x faster than one wide scan)
    u'        = max(u, S)                          1 flat tensor_tensor
dist = rl + hl - u_H[rl]; extraction via one-hot at rl + reduce; the
trivial softmax epilogue runs on host in float64.
"""

import numpy as np
from contextlib import ExitStack

import concourse.bass as bass
import concourse.mybir as mybir
import concourse.tile as tile
from concourse.vector_clock import ScopedClock, VectorClock


def _split_drain_and_barrier(self, tick_clock, wait_clock):
    """Replacement for TileContext._drain_and_barrier: the walrus build in
    this container rejects instructions carrying more than one sync wait,
    so emit one single-wait drain per outstanding proc instead of a single
    drain waiting on every semaphore."""
    gc = tick_clock.global_clock
    nprocs = len(gc)
    for p in range(nprocs):
        t = gc[p]
        if t <= 0:
            continue
        vc = VectorClock([0] * nprocs)
        vc.require_at_least(p, t)
        d = self.nc.sync.drain()
        wait_clock.add_sem_waits(d.ins, ScopedClock({None: vc}))
    self.nc.all_engine_barrier()
    assert self.sems is not None
    popped = self.nc._tile_sem_poison_stack.pop()
    assert popped is self._sem_poison
    self.nc.clear_and_free_semaphores(list(self.sems.allocated().values()))
    self.nc.all_engine_barrier()


tile.TileContext._drain_and_barrier = _split_drain_and_barrier

# Problem constants (hardcoded per contract)
B, S = 128, 64          # batch, samples
RL, H = 256, 256        # ref len, hyp len
NCORES = 8
NPC = (B // NCORES) * S  # 1024 sequences per core
G = NPC // 128           # 8 segments of 128 sequences
SEGW = 260               # cols 0..257 used (j=0..257), 258/259 spacer
W = G * SEGW             # 2080
F16 = mybir.dt.float16
F32 = mybir.dt.float32
AO = mybir.AluOpType
TBUCKET = 16


class _Runner:
    """Compiled SPMD executable for a Bass module (mirrors
    bass2jax.run_bass_via_pjrt, but cached + device-resident timing)."""

    def __init__(self, nc, n_cores):
        import jax
        from jax.sharding import Mesh, PartitionSpec
        from jax.experimental.shard_map import shard_map
        from concourse.bass2jax import (_bass_exec_p, install_neuronx_cc_hook,
                                        partition_id_tensor)

        install_neuronx_cc_hook()
        self.nc, self.n_cores = nc, n_cores
        part_name = (nc.partition_id_tensor.name
                     if nc.partition_id_tensor else None)
        in_names, out_names, out_avals, zero_outs = [], [], [], []
        for alloc in nc.m.functions[0].allocations:
            if not isinstance(alloc, mybir.MemoryLocationSet):
                continue
            name = alloc.memorylocations[0].name
            if alloc.kind == "ExternalInput":
                if name != part_name:
                    in_names.append(name)
            elif alloc.kind == "ExternalOutput":
                out_names.append(name)
                shape = tuple(alloc.tensor_shape)
                dtype = mybir.dt.np(alloc.dtype)
                out_avals.append(jax.core.ShapedArray(shape, dtype))
                zero_outs.append(np.zeros(shape, dtype))
        self.in_names, self.out_names = list(in_names), out_names
        self.out_avals, self.zero_outs = out_avals, zero_outs
        n_params, n_outs = len(in_names), len(out_names)
        all_names = in_names + out_names
        if part_name is not None:
            all_names = all_names + [part_name]

        def _body(*args):
            operands = list(args)
            if part_name is not None:
                operands.append(partition_id_tensor())
            outs = _bass_exec_p.bind(
                *operands,
                out_avals=tuple(out_avals),
                in_names=tuple(all_names),
                out_names=tuple(out_names),
                lowering_input_output_aliases=(),
                sim_require_finite=True,
                sim_require_nnan=True,
                nc=nc,
            )
            return tuple(outs)

        devices = jax.devices()[:n_cores]
        self.mesh = Mesh(np.asarray(devices), ("core",))
        in_specs = (PartitionSpec("core"),) * (n_params + n_outs)
        out_specs = (PartitionSpec("core"),) * n_outs
        self.sharded = jax.jit(
            shard_map(_body, mesh=self.mesh, in_specs=in_specs,
                      out_specs=out_specs, check_rep=False),
            keep_unused=True,
        )
        self.jax = jax
        self._zeros = None

    def place_inputs(self, in_maps):
        import jax
        from jax.sharding import NamedSharding, PartitionSpec
        sh = NamedSharding(self.mesh, PartitionSpec("core"))
        concat = [
            np.concatenate([np.asarray(m[name]) for m in in_maps], axis=0)
            for name in self.in_names
        ]
        return [jax.device_put(a, sh) for a in concat]

    def zeros(self):
        import jax
        from jax.sharding import NamedSharding, PartitionSpec
        sh = NamedSharding(self.mesh, PartitionSpec("core"))
        return [
            jax.device_put(
                np.zeros((self.n_cores * z.shape[0], *z.shape[1:]), z.dtype), sh)
            for z in self.zero_outs
        ]

    def __call__(self, dev_inputs):
        if self._zeros is None:
            self._zeros = self.zeros()
        return self.sharded(*dev_inputs, *self._zeros)

    def gather(self, out_arrs):
        res = []
        for c in range(self.n_cores):
            res.append({
                name: np.asarray(out_arrs[i]).reshape(
                    self.n_cores, *self.out_avals[i].shape)[c]
                for i, name in enumerate(self.out_names)
            })
        return res


def _build_nc(tdrop: tuple):
    """tdrop[g] = step after which segment g is fully frozen (multiple of
    TBUCKET; 256 = never drops). Nondecreasing (globally sorted layout)."""
    WIN = G * RL + 2 * G * H + G + SEGW
    nc = bass.Bass()
    inp = nc.declare_dram_parameter("inp", [128, WIN], F32, isOutput=False)
    chn = nc.declare_dram_parameter("chn", [128, G], F32, isOutput=False)
    y_out = nc.declare_dram_parameter("yrm", [128, G], F32, isOutput=True)

    with ExitStack() as ctx:
        tc = ctx.enter_context(tile.TileContext(nc))
        pool = ctx.enter_context(tc.tile_pool(name="main", bufs=1))

        inp_sb = pool.tile([128, WIN], F32, tag="inp")
        nc.sync.dma_start(inp_sb[:], inp[:])
        # tiny chain input: lets the host time back-to-back executions with
        # a data dependency (output feeds next call's chn), no host overhead
        chn_sb = pool.tile([128, G], F32, tag="chn")
        nc.sync.dma_start(chn_sb[:], chn[:])
        ref_f32 = inp_sb[:, 0:G * RL]
        o1 = G * RL
        hyp3 = inp_sb[:, o1:o1 + G * H].rearrange("p (g t) -> p g t",
                                                  g=G, t=H)
        o2 = o1 + G * H
        act3 = inp_sb[:, o2:o2 + G * H].rearrange("p (g t) -> p g t",
                                                  g=G, t=H)
        o3 = o2 + G * H
        rl_sb = inp_sb[:, o3:o3 + G]
        iota = inp_sb[:, o3 + G:o3 + G + SEGW]

        # ref cast to fp16, one 256-col block per SEGW segment (cols 0..255
        # of the eq1 layout, which stores eq1 for j = c+1)
        REF = pool.tile([128, W], F16, tag="ref")
        R3 = REF[:].rearrange("p (g c) -> p g c", g=G, c=SEGW)
        nc.vector.tensor_copy(
            R3[:, :, 0:RL],
            ref_f32.rearrange("p (g c) -> p g c", g=G, c=RL))

        # DP state and temporaries (eq1 double-buffered per 8-step chunk)
        CH = 8
        U1 = pool.tile([128, W], F16, tag="u1")
        U2 = pool.tile([128, W], F16, tag="u2")
        EK = [pool.tile([128, W], F16, tag=f"ek{i}", name=f"ek{i}")
              for i in range(2 * CH)]
        AZ = pool.tile([128, W], F16, tag="az")
        Z0 = pool.tile([128, W], F16, tag="z0")
        RM = pool.tile([128, W], F16, tag="rm")
        yrm_sb = pool.tile([128, G], F32, tag="yrm")

        for t_ in (U1, U2, Z0, AZ):
            nc.vector.memset(t_[:], 0.0)
        # poison eq1 spacer cols so the flat shifted add leaves
        # A[g][0] <= -1024 at every segment head (scan boundary)
        for Et in EK:
            nc.vector.memset(Et[:], 0.0)
            E3k = Et[:].rearrange("p (g c) -> p g c", g=G, c=SEGW)
            nc.vector.memset(E3k[:, :, RL:SEGW], -2048.0)

        # extraction one-hot at col rl per segment (RM[g, j] = (j == rl))
        RM3 = RM[:].rearrange("p (g c) -> p g c", g=G, c=SEGW)
        nc.vector.memset(RM[:], 0.0)
        for g in range(G):
            nc.vector.tensor_scalar(RM3[:, g, 0:SEGW], iota,
                                    rl_sb[:, g:g + 1], None, AO.is_equal)

        def active_g0(t):
            g0 = 0
            while g0 < G and tdrop[g0] < t:
                g0 += 1
            return g0

        def emit_ts_burst(buf, t0):
            """eq1 for steps t0..t0+CH-1 into EK[buf*CH:] (independent)."""
            for k in range(CH):
                t = t0 + k
                if t > H:
                    break
                E3k = EK[buf * CH + k][:].rearrange("p (g c) -> p g c",
                                                    g=G, c=SEGW)
                for g in range(active_g0(t), G):
                    nc.vector.tensor_scalar(
                        E3k[:, g, 0:RL], R3[:, g, 0:RL],
                        hyp3[:, g, t - 1:t], act3[:, g, t - 1:t],
                        AO.is_equal, AO.add)

        emit_ts_burst(0, 1)
        Uc, Vc = U1, U2
        for t in range(1, H + 1):
            k = (t - 1) % CH
            if k == 0 and t + CH <= H + 1:
                emit_ts_burst(1 - ((t - 1) // CH) % 2, t + CH)
            Ek = EK[(((t - 1) // CH) % 2) * CH + k][:]
            g0 = active_g0(t)
            # 4 streams of 2 segments, phases grouped (g4 schedule):
            # stream deps land >=3 blocks back, hiding drain latency
            streams = []
            for s in range(4):
                lo = max(2 * s, g0)
                if lo < 2 * s + 2:
                    streams.append((lo * SEGW, (2 * s + 2) * SEGW))
            # A[j] = u[j-1] + eq1[j]
            for (lo, hi) in streams:
                nc.vector.tensor_tensor(
                    AZ[:, lo + 1:hi], Uc[:, lo:hi - 1], Ek[:, lo:hi - 1],
                    AO.add)
            # S_g = cummax(A_g) over j = 0..257 (col0 <= -1024 resets)
            for g in range(g0, G):
                nc.vector.tensor_tensor_scan(
                    Vc[:, g * SEGW:g * SEGW + RL + 2],
                    Z0[:, g * SEGW:g * SEGW + RL + 2],
                    AZ[:, g * SEGW:g * SEGW + RL + 2],
                    0.0, AO.add, AO.max)
            # u' = max(u, S)
            for (lo, hi) in streams:
                nc.vector.tensor_tensor(
                    Vc[:, lo:hi], Vc[:, lo:hi], Uc[:, lo:hi], AO.max)
            Uc, Vc = Vc, Uc

        # extraction: u_H at col rl per segment (u stored at col j)
        nc.vector.tensor_tensor(AZ[:], Uc[:], RM[:], AO.mult)
        A3 = AZ[:].rearrange("p (g c) -> p g c", g=G, c=SEGW)
        for g in range(G):
            nc.vector.tensor_reduce(yrm_sb[:, g:g + 1], A3[:, g, :],
                                    mybir.AxisListType.X, AO.add)
        nc.sync.dma_start(y_out[:], yrm_sb[:])

    return nc


_NC_CACHE = {}


def _get_nc(tdrop):
    if tdrop not in _NC_CACHE:
        _NC_CACHE[tdrop] = _build_nc(tdrop)
    return _NC_CACHE[tdrop]


_RUNNER_CACHE = {}


def _get_runner(nc):
    key = id(nc)
    if key not in _RUNNER_CACHE:
        _RUNNER_CACHE[key] = _Runner(nc, NCORES)
    return _RUNNER_CACHE[key]


def _lens(tok, axis):
    is_eos = tok == 0
    has = is_eos.any(axis=axis)
    idx = np.argmax(is_eos, axis=axis)
    return np.where(has, idx + 1, tok.shape[axis])


def _prep(log_probs, ref, hyp):
    """Global sort by hyp length; build per-core DMA images."""
    refT = np.ascontiguousarray(ref.T).astype(np.float32)       # (B, RL)
    # (B*S, H) with sequence index s = b*S + k (matches reference flatten)
    hypF = np.ascontiguousarray(
        hyp.transpose(1, 2, 0)).reshape(B * S, H).astype(np.float32)
    hls = _lens(hypF, 1).astype(np.int64)                       # (B*S,)
    rlens = _lens(np.asarray(ref), 0).astype(np.int64)          # (B,)

    order = np.argsort(hls, kind="stable")                      # global sort
    hyp_s = hypF[order]
    hls_s = hls[order]
    rl_s = np.repeat(rlens, S)[order].astype(np.float32)
    ref_s = refT[order // S]                                    # (B*S, RL)

    # pad ended hyp positions with a never-matching token
    tidx = np.arange(H)[None, :]
    act_s = (tidx < hls_s[:, None]).astype(np.float32)
    hyp_s = np.where(act_s > 0, hyp_s, -1.0).astype(np.float32)

    iota = np.broadcast_to(np.arange(SEGW, dtype=np.float32), (128, SEGW))
    in_maps = []
    for c in range(NCORES):
        idx = np.arange(c, B * S, NCORES)                       # ranks of core c
        # rank r -> seg = (r//8)//128, row = (r//8)%128
        q = idx // NCORES
        segs, rows = q // 128, q % 128
        refc = np.zeros((128, G, RL), np.float32)
        hypc = np.zeros((128, G, H), np.float32)
        actc = np.zeros((128, G, H), np.float32)
        rlc = np.zeros((128, G), np.float32)
        refc[rows, segs] = ref_s[idx]
        hypc[rows, segs] = hyp_s[idx]
        actc[rows, segs] = act_s[idx]
        rlc[rows, segs] = rl_s[idx]
        in_maps.append({"inp": np.ascontiguousarray(np.concatenate(
            [refc.reshape(128, G * RL), hypc.reshape(128, G * H),
             actc.reshape(128, G * H), rlc, iota], axis=1)),
            "chn": np.zeros((128, G), np.float32)})
    seg_max = np.zeros(G, dtype=np.int64)
    hq = hls_s.reshape(G, 128 * NCORES)
    seg_max = hq.max(axis=1)
    tdrop = tuple(int(min(H, -(-m // TBUCKET) * TBUCKET)) for m in seg_max)
    return in_maps, order, tdrop


def _epilogue(youts, order, log_probs, ref, hyp):
    ref_np = np.asarray(ref)
    rlens_b = _lens(ref_np, 0).astype(np.float64)               # (B,)
    hypF = np.asarray(hyp).transpose(1, 2, 0).reshape(B * S, H)
    hls = _lens(hypF, 1).astype(np.float64)                     # (B*S,)
    rl_all = np.repeat(rlens_b, S)                              # (B*S,)

    u_final = np.empty(B * S, dtype=np.float64)
    for c in range(NCORES):
        idx = np.arange(c, B * S, NCORES)
        q = idx // NCORES
        segs, rows = q // 128, q % 128
        u_final[order[idx]] = youts[c][rows, segs]
    dist = rl_all + hls - u_final
    er = (dist / rl_all).reshape(B, S)
    er = er - er.mean(axis=1, keepdims=True)
    lp = np.asarray(log_probs).astype(np.float64)
    sm = np.exp(lp - lp.max(axis=1, keepdims=True))
    sm /= sm.sum(axis=1, keepdims=True)
    return np.float32((er * sm).mean())


def kernel(log_probs, ref, hyp, _sim=False):
    lp, ref, hyp = np.asarray(log_probs), np.asarray(ref), np.asarray(hyp)
    in_maps, order, tdrop = _prep(lp, ref, hyp)
    nc = _get_nc(tdrop)
    if _sim:
        from concourse.bass_interp import CoreSim
        youts = []
        for c in range(NCORES):
            sim = CoreSim(nc, trace=False)
            for k, v in in_maps[c].items():
                sim.tensor(k)[:] = v
            sim.simulate()
            youts.append(np.array(sim.tensor("yrm")))
        return _epilogue(youts, order, lp, ref, hyp)

    runner = _get_runner(nc)
    dev_in = runner.place_inputs(in_maps)
    results = runner.gather(runner(dev_in))
    youts = [results[c]["yrm"] for c in range(NCORES)]
    return _epilogue(youts, order, lp, ref, hyp)
